# revision 1
# baseline (speedup 1.0000x reference)
"""Trainium2 Bass kernel for CurveGraphic2d (bezier curve rendering).

Computes, for B=32 cubic bezier curves, a 256x256 canvas per curve:
    canvas[b, y, x] = 1 - (min_s ||p - s_bs|| / 4 + 1e-6)^0.35
where s_bs are 32 samples along curve b.

Sharding: data-parallel over PIXELS across 8 cores (8192 pixels per core,
all 32 curves on every core).  Each core computes its [8192, 32*32]
squared-distance matrix on TensorE via the GEMM trick

    d2[p,(b,s)] = y*(-2sy) + x*(-2sx) + p2*1 + 1*s2

with every operand expressed as a hi+lo pair of float32r values (fp32r =
fp32 RNE-rounded to 11 explicit mantissa bits; the hi/lo split is exact, so
the K=8 fp32r matmul reproduces fp32-quality d2 while streaming at 4x the
fp32 rate).  VectorE min-reduces over the 32 samples per curve, and ScalarE
applies the canvas tail without the low-precision Sqrt LUT:

    canvas = 1 - exp(0.175*ln(relu(d2min) + 1.6e-11) - 0.35*ln 4)

Pixel mapping per core c: global n = c*8192 + p*64 + t  (p = SBUF
partition, t = local pixel-tile 0..63).  Four pixel-tiles run as concurrent
matmuls in distinct PE row-groups (tile_position=(32g, 0)).
"""

import numpy as np
from math import comb, log as _ln

H, W = 256, 256
S = 32
K = 4
B = 32
NCORES = 8
N = H * W                     # 65536 pixels
NLOC = N // NCORES            # 8192 pixels per core
TLOC = NLOC // 128            # 64 local pixel tiles
NGROUP = TLOC // 4            # 16 groups of 4 strips
BS = B * S                    # 1024 (curve, sample) columns
WIDTH = 4.0
AAF = 0.35
LN_BIAS = 1.6e-11             # ~ (4*eps)^2: matches reference's +eps at d2=0
EXP_BIAS = -AAF * _ln(WIDTH)  # -0.35 * ln(4)

_PROG = None


def _bernstein_basis(num_samples, k):
    ts = np.linspace(0.0, 1.0, num_samples, dtype=np.float32)
    i = np.arange(k, dtype=np.float32)
    binom = np.array([comb(k - 1, j) for j in range(k)], dtype=np.float32)
    return (binom * ts[:, None] ** i * (1.0 - ts[:, None]) ** (k - 1 - i)).astype(
        np.float32
    )


NCONV = 23        # of the 32 strip-pairs, how many take the ACT+bf16 path


def _pair_kinds():
    """kinds[i] for pair i = 0..31: True = ACT relu+bf16 convert + DVE
    tensor_tensor tree, False = direct fp32 tensor_reduce.  Evenly
    interleaved so ACT and DVE run concurrently through the whole loop,
    with conv pairs first/last so ACT starts early and direct-region tail
    work can overlap the final converts."""
    kinds = [((i + 1) * NCONV) // 32 - (i * NCONV) // 32 == 1 for i in range(32)]
    assert sum(kinds) == NCONV
    return kinds


def _tile_perm():
    """t_global[i][e] for pair i, member e: logical canvas tile index.
    bf16 pairs fill t in [0, 2*NCONV), direct pairs fill [2*NCONV, 64)."""
    kinds = _pair_kinds()
    tconv = 2 * sum(kinds)
    perm = []
    ncv = ndr = 0
    for i in range(32):
        if kinds[i]:
            perm.append((2 * ncv, 2 * ncv + 1))
            ncv += 1
        else:
            perm.append((tconv + 2 * ndr, tconv + 2 * ndr + 1))
            ndr += 1
    return perm


def _round_f32r(x):
    """fp32 -> nearest fp32r (11 explicit mantissa bits), bit-exact to HW."""
    u = np.asarray(x, np.float32).view(np.uint32).astype(np.uint64)
    u = (u + np.uint64(1 << 11)) & np.uint64(0xFFFFF000)
    return (u & np.uint64(0xFFFFFFFF)).astype(np.uint32).view(np.float32)


def _hi_lo(x):
    x = np.asarray(x, np.float32)
    hi = _round_f32r(x)
    lo = _round_f32r(x - hi)
    return hi, lo


def _build_bass():
    import concourse.mybir as mybir
    import concourse.tile as tile
    from concourse import bacc

    f32 = mybir.dt.float32
    f32r = mybir.dt.float32r
    bf16 = mybir.dt.bfloat16
    AF = mybir.ActivationFunctionType
    nc = bacc.Bacc("TRN2")

    # activation() lowers non-Copy biases through the const-AP database;
    # register the two bias constants the tail needs
    for val in (LN_BIAS, EXP_BIAS):
        cst = nc.alloc_sbuf_tensor(f"const-f32-{val}", [128, 1], f32)
        nc.gpsimd.memset(cst.ap(), val)
        nc.const_aps.aps[(f32, val)] = cst.ap()
    nc.all_engine_barrier()

    p32 = nc.dram_tensor("p32", [32, NGROUP * 128], f32r, kind="ExternalInput")
    sall = nc.dram_tensor("sall", [8, BS], f32r, kind="ExternalInput")
    out = nc.dram_tensor("out", [B, NLOC], f32, kind="ExternalOutput")

    with tile.TileContext(nc) as tc:
        with (
            tc.tile_pool(name="sb", bufs=1) as sb,
            tc.tile_pool(name="ps", bufs=1, space="PSUM") as pp,
        ):
            ppack = sb.tile([128, NGROUP * 128], f32r)
            sreps = sb.tile([128, BS], f32r)
            # NCONV of the 32 strip-pairs go through an ACT relu+bf16 convert
            # and a DVE tensor_tensor min-tree (2x bf16 mode); the rest are
            # direct fp32 tensor_reduce from PSUM.  The pattern is interleaved
            # across the loop so ACT and DVE stay concurrently busy; the
            # host-side pixel permutation (_pair_kinds) keeps each dtype's
            # minstrip region contiguous.
            kinds = _pair_kinds()
            TCONV = 2 * sum(kinds)                      # t < TCONV is bf16
            TDIR = TLOC - TCONV
            minstrip16 = sb.tile([128, B * TCONV], bf16)  # col = TCONV*b + t
            minstrip = sb.tile([128, B * TDIR], f32)      # col = TDIR*b + (t-TCONV)
            canvas = sb.tile([128, B * TLOC], f32)        # col = 64*b + t

            # input DMAs: per-strip, split in two column pieces for early start
            half = (NGROUP * 128) // 2
            for g in range(4):
                for q in range(2):
                    nc.sync.dma_start(
                        ppack[32 * g : 32 * g + 8, q * half : (q + 1) * half],
                        p32[8 * g : 8 * g + 8, q * half : (q + 1) * half],
                    )
                nc.sync.dma_start(sreps[32 * g : 32 * g + 8, :], sall[:, :])

            # strips are processed in pairs sharing one 4-bank PSUM tile
            ncv = 0
            ndr = 0
            for u in range(NGROUP):
                for a in range(2):                       # strip pair (2a, 2a+1)
                    d2 = pp.tile([128, 2 * BS], f32, name=f"d2_{a}", tag=f"d2_{a}")
                    for e in range(2):                   # strip g = 2a + e
                        g = 2 * a + e
                        for h in range(2):
                            nc.tensor.matmul(
                                d2[:, 1024 * e + 512 * h : 1024 * e + 512 * (h + 1)],
                                ppack[32 * g : 32 * g + 8, u * 128 : (u + 1) * 128],
                                sreps[32 * g : 32 * g + 8, 512 * h : 512 * (h + 1)],
                                start=True,
                                stop=True,
                                tile_position=(32 * g, 0),
                            )
                    conv = kinds[2 * u + a]
                    if conv:
                        t0 = 2 * ncv                     # rank in bf16 region
                        ncv += 1
                    else:
                        t0 = 2 * ndr                     # rank in f32 region
                        ndr += 1
                    if conv:
                        # ACT: relu + bf16 convert (min(relu(x)) == relu(min(x)))
                        bc = sb.tile([128, 2 * BS], bf16, name="bc", tag="bc",
                                     bufs=4)
                        nc.scalar.activation(bc[:, :], d2[:, :], AF.Relu)
                        # DVE bf16 min-tree over s (2x_1P mode)
                        src = bc.rearrange("p (e b s) -> p e b s", b=B, s=S)
                        cur, width = src, S
                        for lvl in range(4):
                            width //= 2
                            nxt = sb.tile(
                                [128, 2 * B * width], bf16,
                                name=f"tr{lvl}", tag=f"tr{lvl}", bufs=2,
                            ).rearrange("p (e b s) -> p e b s", b=B, s=width)
                            nc.vector.tensor_tensor(
                                nxt, cur[:, :, :, :width], cur[:, :, :, width:],
                                op=mybir.AluOpType.min,
                            )
                            cur = nxt
                        outv = (
                            minstrip16.rearrange("p (b t) -> p t b", t=TCONV)
                            [:, t0 : t0 + 2, :]
                        )
                        nc.vector.tensor_tensor(
                            outv, cur[:, :, :, 0], cur[:, :, :, 1],
                            op=mybir.AluOpType.min,
                        )
                    else:
                        inv = d2.rearrange("p (e b s) -> p e b s", b=B, s=S)
                        outv = (
                            minstrip.rearrange("p (b t) -> p t b", t=TDIR)
                            [:, t0 : t0 + 2, :]
                        )
                        nc.vector.tensor_reduce(
                            outv, inv, axis=mybir.AxisListType.X,
                            op=mybir.AluOpType.min,
                        )

            # tail: canvas = 1 - exp(0.175*ln(relu(d2min) + 1.6e-11) - 0.35*ln4)
            # (bf16 region is already relu'd by the convert; the direct
            # region's relu and the final 1-x run on DVE tensor_scalar to
            # keep ScalarE on the transcendentals only)
            cbt = canvas.rearrange("p (b t) -> p b t", t=TLOC)
            mv = minstrip.rearrange("p (b t) -> p b t", t=TDIR)
            nc.scalar.activation(cbt[:, :, TCONV:], mv, AF.Relu)
            m16 = minstrip16.rearrange("p (b t) -> p b t", t=TCONV)
            th = TCONV // 2
            nc.scalar.activation(cbt[:, :, :th], m16[:, :, :th], AF.Ln,
                                 bias=LN_BIAS, scale=1.0)
            nc.scalar.activation(cbt[:, :, th:TCONV], m16[:, :, th:], AF.Ln,
                                 bias=LN_BIAS, scale=1.0)
            nc.scalar.activation(cbt[:, :, TCONV:], cbt[:, :, TCONV:], AF.Ln,
                                 bias=LN_BIAS, scale=1.0)
            NW = 4
            wl = (B * TLOC) // NW
            bwv = [canvas[:, w * wl : (w + 1) * wl] for w in range(NW)]
            for cv in bwv:
                nc.scalar.activation(cv, cv, AF.Exp, scale=AAF / 2.0, bias=EXP_BIAS)
            for cv in bwv:
                nc.vector.tensor_scalar(
                    cv, cv, -1.0, 1.0,
                    op0=mybir.AluOpType.mult, op1=mybir.AluOpType.add,
                )

            # output DMAs per b-chunk wave: canvas[p, 64b+t] -> out[b, p*64+t]
            bpw = B // NW
            for w in range(NW):
                dst = out[w * bpw : (w + 1) * bpw, :].rearrange(
                    "b (p t) -> p b t", t=TLOC
                )
                src = canvas[:, w * wl : (w + 1) * wl].rearrange(
                    "p (b t) -> p b t", t=TLOC
                )
                nc.sync.dma_start(dst, src)
    nc.compile()
    return nc


def _get_prog():
    global _PROG
    if _PROG is None:
        _PROG = _build_bass()
    return _PROG


def _host_prep(inputs):
    """Returns (p32_list per-core [32, 2048], sall [128, 1024] shared)."""
    inp = np.asarray(inputs, dtype=np.float32)           # [B, K, 2] in [0,1]
    kp = inp * np.array([H, W], dtype=np.float32)
    basis = _bernstein_basis(S, K)
    samples = np.einsum("sk,bkd->bsd", basis, kp).astype(np.float32)  # [B, S, 2]

    sy = samples[..., 0].reshape(-1)                     # [BS] b-major
    sx = samples[..., 1].reshape(-1)
    s2 = (sy * sy + sx * sx).astype(np.float32)
    cyh, cyl = _hi_lo(-2.0 * sy)
    cxh, cxl = _hi_lo(-2.0 * sx)
    s2h, s2l = _hi_lo(s2)
    ones = np.ones(BS, np.float32)
    sall = np.ascontiguousarray(
        np.stack([cyh, cyl, cxh, cxl, ones, ones, s2h, s2l])
    )  # [8, BS]

    # per-core pixel features: P32[8g+k, u*128 + m] = feat_k(l = m*64 + t)
    # where t = _tile_perm()[2u + g//2][g % 2] (pair interleaving permutation)
    perm = _tile_perm()
    t_of = np.zeros((4, NGROUP), dtype=np.int64)         # [g, u]
    for u in range(NGROUP):
        for g in range(4):
            t_of[g, u] = perm[2 * u + g // 2][g % 2]
    m_idx = np.arange(128)
    l = m_idx[None, None, :] * TLOC + t_of[:, :, None]   # [4, 16, 128]
    p32s = []
    for c in range(NCORES):
        n = c * NLOC + l
        y = (n // W).astype(np.int64)
        x = (n % W).astype(np.int64)
        p2 = (y * y + x * x).astype(np.float32)
        p2h, p2l = _hi_lo(p2)
        onesf = np.ones_like(p2, dtype=np.float32)
        feats = np.stack(
            [y.astype(np.float32), y.astype(np.float32),
             x.astype(np.float32), x.astype(np.float32),
             p2h, p2l, onesf, onesf], axis=1)            # [4g, 8k, 16u, 128m]
        p32s.append(np.ascontiguousarray(feats.reshape(32, NGROUP * 128)))
    return p32s, sall


def _run(inputs, trace=False):
    from concourse.bass_utils import run_bass_kernel_spmd

    p32s, sall = _host_prep(inputs)
    nc = _get_prog()
    in_maps = [{"p32": p32s[c], "sall": sall} for c in range(NCORES)]
    res = run_bass_kernel_spmd(
        nc, in_maps, core_ids=list(range(NCORES)), trace=trace
    )
    # core c's out is [B, 8192] covering pixels [c*8192, (c+1)*8192)
    full = np.concatenate(
        [res.results[c]["out"] for c in range(NCORES)], axis=1
    ).reshape(B, H, W).astype(np.float32)
    return full, res


def kernel(**inputs):
    full, _ = _run(inputs["inputs"], trace=False)
    return full


# ---------------- benchmarking helpers (not used by the grader) -------------


def _make_jitted(nc):
    """Build the sharded jit callable once (mirrors run_bass_via_pjrt)."""
    import jax
    import concourse.mybir as mybir
    from jax.sharding import Mesh, PartitionSpec
    from jax.experimental.shard_map import shard_map
    from concourse.bass2jax import _bass_exec_p, install_neuronx_cc_hook, partition_id_tensor

    install_neuronx_cc_hook()
    partition_name = nc.partition_id_tensor.name if nc.partition_id_tensor else None
    in_names, out_names, out_avals, zero_outs = [], [], [], []
    for alloc in nc.m.functions[0].allocations:
        if not isinstance(alloc, mybir.MemoryLocationSet):
            continue
        name = alloc.memorylocations[0].name
        if alloc.kind == "ExternalInput":
            if name != partition_name:
                in_names.append(name)
        elif alloc.kind == "ExternalOutput":
            shape = tuple(alloc.tensor_shape)
            dtype = mybir.dt.np(alloc.dtype)
            out_names.append(name)
            out_avals.append(jax.core.ShapedArray(shape, dtype))
            zero_outs.append(np.zeros(shape, dtype))
    n_params = len(in_names)
    n_outs = len(out_avals)
    all_in = list(in_names) + list(out_names)
    if partition_name is not None:
        all_in.append(partition_name)

    def _body(*args):
        operands = list(args)
        if partition_name is not None:
            operands.append(partition_id_tensor())
        outs = _bass_exec_p.bind(
            *operands,
            out_avals=tuple(out_avals),
            in_names=tuple(all_in),
            out_names=tuple(out_names),
            lowering_input_output_aliases=(),
            sim_require_finite=True,
            sim_require_nnan=True,
            nc=nc,
        )
        return tuple(outs)

    devices = jax.devices()[:NCORES]
    mesh = Mesh(np.asarray(devices), ("core",))
    in_specs = (PartitionSpec("core"),) * (n_params + n_outs)
    out_specs = (PartitionSpec("core"),) * n_outs
    fn = jax.jit(
        shard_map(_body, mesh=mesh, in_specs=in_specs, out_specs=out_specs,
                  check_rep=False),
        keep_unused=True,
    )
    return fn, in_names, out_names, zero_outs


def bench(inputs, iters=30):
    """Returns (output, per-call seconds list) using a cached jitted callable."""
    import jax
    import time

    p32s, sall = _host_prep(inputs)
    nc = _get_prog()
    fn, in_names, out_names, zero_outs = _make_jitted(nc)
    concat_in = []
    for name in in_names:
        if name == "p32":
            concat_in.append(np.concatenate(p32s, axis=0))
        elif name == "sall":
            concat_in.append(np.concatenate([sall] * NCORES, axis=0))
        else:
            raise KeyError(name)
    for z in zero_outs:
        concat_in.append(np.concatenate([z] * NCORES, axis=0))
    args = [jax.device_put(a) for a in concat_in]
    out = fn(*args)
    jax.block_until_ready(out)
    times = []
    for _ in range(iters):
        t0 = time.perf_counter()
        out = fn(*args)
        jax.block_until_ready(out)
        times.append(time.perf_counter() - t0)
    arr = np.asarray(out[0])                      # [8*B, NLOC]
    parts = [arr[c * B : (c + 1) * B] for c in range(NCORES)]
    full = np.concatenate(parts, axis=1).reshape(B, H, W).astype(np.float32)
    return full, times



# revision 11
# speedup vs baseline: 1.0624x; 1.0624x over previous
"""Trainium2 Bass kernel for CurveGraphic2d (bezier curve rendering).

Computes, for B=32 cubic bezier curves, a 256x256 canvas per curve:
    canvas[b, y, x] = 1 - (min_s ||p - s_bs|| / 4 + 1e-6)^0.35
where s_bs are 32 samples along curve b.

Sharding: data-parallel over PIXELS across 8 cores (8192 pixels per core,
all 32 curves on every core).  Each core computes its [8192, 32*32]
squared-distance matrix on TensorE via the GEMM trick

    d2[p,(b,s)] = y*(-2sy) + x*(-2sx) + p2*1 + 1*s2

with every operand a hi+lo pair of float32r values (exact split, K=8).

Hardware constraints pin the PSUM crossing to ACT and DVE (one PSUM
operand max per DVE op; GPSIMD can't reach PSUM or run min at all), so
the 64 pixel-strips are processed as 32 strip-pairs split:

  - 23 "A" pairs: ACT reads d2 from PSUM and writes ln(d2+0.04) as bf16 in
    ONE pass -- ln is monotone (min ln = ln min), so the canvas tail's Ln
    rides the crossing for free, and the +0.04 bias clamps fp-negative d2
    (no relu pass anywhere).  DVE reduces in bf16 (2:1 then rank-batched
    trees, all levels in 2x mode via an s-outermost layout).
  -  9 "F" pairs: a single DVE tensor_reduce min straight from PSUM to the
    bf16 min strip (raw domain; Ln applied to the reduced strip early).

The tail is exp(0.175*L - 0.35 ln 4) on ACT + (1-x) on DVE + chunked
output DMA.  Raw-region Lns complete mid-loop, so the ONE Ln->Exp
activation-table switch (the tables share no set with both) stays off the
critical path; the last slots are all A-class (exp-only), leaving a short
tree+exp+sub+DMA chain to drain.  Input DMAs are ordered so the first
matmul starts ~2us in (small leading pieces), and exp chunks shrink
toward the end.
"""

import numpy as np
from math import comb, log as _ln

H, W = 256, 256
S = 32
K = 4
B = 32
NCORES = 8
N = H * W                     # 65536 pixels
NLOC = N // NCORES            # 8192 pixels per core
TLOC = NLOC // 128            # 64 local pixel tiles
NGROUP = TLOC // 4            # 16 groups of 4 strips
BS = B * S                    # 1024 (curve, sample) columns
WIDTH = 4.0
AAF = 0.35
LNB = 0.04                    # ln bias: clamps fp-negative d2, tiny distortion
ESC = AAF / 2.0               # 0.175, applied inside Exp
EXP_BIAS = -AAF * _ln(WIDTH)  # -0.35 * ln(4)

# class per pair slot (arrival order): 9 F spread through slots 0..26,
# A elsewhere; the last 5 slots are A (exp-only end).
F_SLOTS = {2, 5, 8, 11, 14, 17, 20, 23, 26}
CLS = ["F" if i in F_SLOTS else "A" for i in range(32)]
NA, NF = CLS.count("A"), CLS.count("F")
assert (NA, NF) == (23, 9)

_PROG = None


def _bernstein_basis(num_samples, k):
    ts = np.linspace(0.0, 1.0, num_samples, dtype=np.float32)
    i = np.arange(k, dtype=np.float32)
    binom = np.array([comb(k - 1, j) for j in range(k)], dtype=np.float32)
    return (binom * ts[:, None] ** i * (1.0 - ts[:, None]) ** (k - 1 - i)).astype(
        np.float32
    )


def _ranks():
    """R[i]: canvas rank of pair slot i.  Rank blocks: A [0,23), F [23,32),
    each in arrival order.  Pair rank r covers canvas tiles (2r, 2r+1)."""
    ra = rf = 0
    R = []
    for c in CLS:
        if c == "A":
            R.append(ra)
            ra += 1
        else:
            R.append(NA + rf)
            rf += 1
    return R


def _batches():
    """A-pair tree batches: consecutive ranks, 5 x G4 + 1 x G3."""
    a_slots = [i for i in range(32) if CLS[i] == "A"]
    return [a_slots[b : b + 4] for b in range(0, NA, 4)]


def _round_f32r(x):
    """fp32 -> nearest fp32r (11 explicit mantissa bits), bit-exact to HW."""
    u = np.asarray(x, np.float32).view(np.uint32).astype(np.uint64)
    u = (u + np.uint64(1 << 11)) & np.uint64(0xFFFFF000)
    return (u & np.uint64(0xFFFFFFFF)).astype(np.uint32).view(np.float32)


def _hi_lo(x):
    x = np.asarray(x, np.float32)
    hi = _round_f32r(x)
    lo = _round_f32r(x - hi)
    return hi, lo


def _build_bass():
    import concourse.mybir as mybir
    import concourse.tile as tile
    from concourse import bacc

    f32 = mybir.dt.float32
    f32r = mybir.dt.float32r
    bf16 = mybir.dt.bfloat16
    AF = mybir.ActivationFunctionType
    MIN = mybir.AluOpType.min
    nc = bacc.Bacc("TRN2")

    # activation() lowers non-Copy biases through the const-AP database
    for val in (LNB, EXP_BIAS):
        cst = nc.alloc_sbuf_tensor(f"const-f32-{val}", [128, 1], f32)
        nc.gpsimd.memset(cst.ap(), val)
        nc.const_aps.aps[(f32, val)] = cst.ap()
    nc.all_engine_barrier()

    p32 = nc.dram_tensor("p32", [32, NGROUP * 128], f32r, kind="ExternalInput")
    sall = nc.dram_tensor("sall", [8, BS], f32r, kind="ExternalInput")
    out = nc.dram_tensor("out", [B, NLOC], f32, kind="ExternalOutput")

    R = _ranks()
    batches = _batches()
    slot_batch = {}
    for bi, slots in enumerate(batches):
        for j, s in enumerate(slots):
            slot_batch[s] = (bi, j)
    f_slots = sorted(F_SLOTS)
    f_chunks = [f_slots[:5], f_slots[5:]]   # lnR in two pieces

    with tile.TileContext(nc) as tc:
        with (
            tc.tile_pool(name="sb", bufs=1) as sb,
            tc.tile_pool(name="ps", bufs=1, space="PSUM") as pp,
        ):
            ppack = sb.tile([128, NGROUP * 128], f32r)
            sreps = sb.tile([128, BS], f32r)
            mins = sb.tile([128, B * TLOC], bf16)   # col = rank*64 + e*32 + b
            canv = sb.tile([128, B * TLOC], f32)    # col = b*64 + t
            canv_bt = canv.rearrange("p (b t) -> p b t", t=TLOC)

            # input DMAs: small leading pieces first so matmuls start early
            lead = 512
            for g in range(4):
                nc.sync.dma_start(
                    ppack[32 * g : 32 * g + 8, :lead],
                    p32[8 * g : 8 * g + 8, :lead],
                )
                nc.sync.dma_start(sreps[32 * g : 32 * g + 8, :], sall[:, :])
            for g in range(4):
                nc.sync.dma_start(
                    ppack[32 * g : 32 * g + 8, lead:],
                    p32[8 * g : 8 * g + 8, lead:],
                )

            m1_tiles = {}

            def get_m1(bi):
                if bi not in m1_tiles:
                    G = len(batches[bi])
                    m1_tiles[bi] = sb.tile(
                        [128, 16 * G * 64], bf16,
                        name=f"m1_{bi}", tag=f"m1_{bi % 2}", bufs=2,
                    )
                return m1_tiles[bi]

            def ln_raw(r0, nt):
                """Ln(min + LNB) of raw mins cols [r0*64, r0*64+nt*32) into
                the b-major canvas (strided out; ACT has no perf modes)."""
                src = mins[:, r0 * 64 : r0 * 64 + nt * 32].rearrange(
                    "p (t b) -> p b t", b=B)
                nc.scalar.activation(
                    canv_bt[:, :, 2 * r0 : 2 * r0 + nt],
                    src, AF.Ln, bias=LNB, scale=1.0)

            def emit_tree(bi):
                """Rank-batched bf16 min-tree 16 -> 1 on DVE; the s-outer
                layout keeps every level's operands packed (2x mode)."""
                slots = batches[bi]
                G = len(slots)
                JEB = G * 64
                m1 = get_m1(bi)
                cur = m1.rearrange("p (s q) -> p s q", q=JEB)
                width = 16
                for lvl in range(3):
                    width //= 2
                    nxt = sb.tile(
                        [128, width * JEB], bf16,
                        name=f"tr{lvl}_{bi}", tag=f"tr{lvl}", bufs=2,
                    ).rearrange("p (s q) -> p s q", q=JEB)
                    nc.vector.tensor_tensor(
                        nxt, cur[:, :width, :], cur[:, width:, :], op=MIN)
                    cur = nxt
                r0 = R[slots[0]]
                nc.vector.tensor_tensor(
                    mins[:, r0 * 64 : r0 * 64 + JEB],
                    cur[:, 0, :], cur[:, 1, :], op=MIN)

            # ---------------- pair loop ----------------
            for i in range(32):
                u, a = i // 2, i % 2
                d2 = pp.tile([128, 2 * BS], f32, name=f"d2_{i}",
                             tag=f"d2_{i % 2}")
                for e in range(2):
                    g = 2 * a + e
                    for h in range(2):
                        nc.tensor.matmul(
                            d2[:, 1024 * e + 512 * h : 1024 * e + 512 * (h + 1)],
                            ppack[32 * g : 32 * g + 8, u * 128 : (u + 1) * 128],
                            sreps[32 * g : 32 * g + 8, 512 * h : 512 * (h + 1)],
                            start=True,
                            stop=True,
                            tile_position=(32 * g, 0),
                        )
                if CLS[i] == "F":
                    # one-shot 32->1 min reduce straight from PSUM (raw)
                    r0 = R[i]
                    nc.vector.tensor_reduce(
                        mins[:, r0 * 64 : r0 * 64 + 64].rearrange(
                            "p (q o) -> p q o", o=1),
                        d2.rearrange("p (q s) -> p q s", s=S),
                        axis=mybir.AxisListType.X, op=MIN)
                    for chunk in f_chunks:
                        if chunk[-1] == i:
                            ln_raw(R[chunk[0]], 2 * len(chunk))
                    continue
                d2v = d2.rearrange("p (q s) -> p s q", s=S)   # [p, s32, eb64]
                bi, j = slot_batch[i]
                G = len(batches[bi])
                m1 = get_m1(bi)
                # L1 output view [p, s16, eb64] into batch slot j (s-outer)
                o = m1.rearrange("p (s j q) -> p s j q", j=G, q=64)[:, :, j, :]
                # ACT: ln(d2 + LNB) PSUM->SBUF bf16, written s-outer
                cv = sb.tile([128, 2 * BS], bf16, name=f"cv_{i}",
                             tag="cv", bufs=4)
                cvv = cv.rearrange("p (s q) -> p s q", q=64)
                nc.scalar.activation(cvv, d2v, AF.Ln, bias=LNB, scale=1.0)
                nc.vector.tensor_tensor(o, cvv[:, :16, :], cvv[:, 16:, :],
                                        op=MIN)
                if j == G - 1:
                    emit_tree(bi)

            # ---------------- tail ----------------
            # exp chunks by readiness: F region first (its Ln is done in-
            # loop), then A ranks, final small chunk = last G3 batch.
            # First exp triggers the single table switch; subs on DVE
            # (2x_2p) and per-chunk output DMA.
            chunks = [(NA, 2 * NF), (0, 24), (12, 16), (20, 6)]
            for (r0, nt) in chunks:
                creg = canv_bt[:, :, 2 * r0 : 2 * r0 + nt]
                if r0 < NA:  # A region: exp straight from bf16 ln-mins
                    src = mins[:, r0 * 64 : r0 * 64 + nt * 32].rearrange(
                        "p (t b) -> p b t", b=B)
                else:
                    src = creg
                nc.scalar.activation(creg, src, AF.Exp,
                                     scale=ESC, bias=EXP_BIAS)
                nc.vector.tensor_scalar(
                    creg, creg, -1.0, 1.0,
                    op0=mybir.AluOpType.mult, op1=mybir.AluOpType.add)
                dst = out.rearrange("b (m t) -> m b t", m=128,
                                    t=TLOC)[:, :, 2 * r0 : 2 * r0 + nt]
                nc.sync.dma_start(dst, creg)
    nc.compile()
    return nc


def _get_prog():
    global _PROG
    if _PROG is None:
        _PROG = _build_bass()
    return _PROG


def _host_prep(inputs):
    """Returns (p32_list per-core [32, 2048], sall [8, 1024] shared)."""
    inp = np.asarray(inputs, dtype=np.float32)           # [B, K, 2] in [0,1]
    kp = inp * np.array([H, W], dtype=np.float32)
    basis = _bernstein_basis(S, K)
    samples = np.einsum("sk,bkd->bsd", basis, kp).astype(np.float32)  # [B,S,2]

    sy = samples[..., 0].reshape(-1)                     # [BS] b-major
    sx = samples[..., 1].reshape(-1)
    s2 = (sy * sy + sx * sx).astype(np.float32)
    cyh, cyl = _hi_lo(-2.0 * sy)
    cxh, cxl = _hi_lo(-2.0 * sx)
    s2h, s2l = _hi_lo(s2)
    ones = np.ones(BS, np.float32)
    sall = np.ascontiguousarray(
        np.stack([cyh, cyl, cxh, cxl, ones, ones, s2h, s2l])
    )  # [8, BS]

    # per-core pixel features: P32[8g+k, u*128 + m] = feat_k(l = m*64 + t)
    # where t = 2*R(2u + g//2) + (g%2)  (rank permutation)
    R = _ranks()
    t_of = np.zeros((4, NGROUP), dtype=np.int64)         # [g, u]
    for u in range(NGROUP):
        for g in range(4):
            t_of[g, u] = 2 * R[2 * u + g // 2] + (g % 2)
    m_idx = np.arange(128)
    l = m_idx[None, None, :] * TLOC + t_of[:, :, None]   # [4, 16, 128]
    p32s = []
    for c in range(NCORES):
        n = c * NLOC + l
        y = (n // W).astype(np.int64)
        x = (n % W).astype(np.int64)
        p2 = (y * y + x * x).astype(np.float32)
        p2h, p2l = _hi_lo(p2)
        onesf = np.ones_like(p2, dtype=np.float32)
        feats = np.stack(
            [y.astype(np.float32), y.astype(np.float32),
             x.astype(np.float32), x.astype(np.float32),
             p2h, p2l, onesf, onesf], axis=1)            # [4g, 8k, 16u, 128m]
        p32s.append(np.ascontiguousarray(feats.reshape(32, NGROUP * 128)))
    return p32s, sall


def _run(inputs, trace=False):
    from concourse.bass_utils import run_bass_kernel_spmd

    p32s, sall = _host_prep(inputs)
    nc = _get_prog()
    in_maps = [{"p32": p32s[c], "sall": sall} for c in range(NCORES)]
    res = run_bass_kernel_spmd(
        nc, in_maps, core_ids=list(range(NCORES)), trace=trace
    )
    # core c's out is [B, 8192] covering pixels [c*8192, (c+1)*8192)
    full = np.concatenate(
        [res.results[c]["out"] for c in range(NCORES)], axis=1
    ).reshape(B, H, W).astype(np.float32)
    return full, res


def kernel(**inputs):
    full, _ = _run(inputs["inputs"], trace=False)
    return full
